# revision 1
# baseline (speedup 1.0000x reference)
"""AVLnet baseline model Bass kernel for 8x TRN2 NeuronCores (v2).

Contract: kernel(**inputs) takes the FULL (unsharded) numpy inputs as produced
by setup_inputs() and returns the full (3, 32, 4096) float32 output.

Strategy vs v1 baseline:
- Ragged conv lengths: samples are sorted by audio_STFT_nframes and assigned
  to (core, slot) so each compiled slot only computes the valid prefix of the
  conv chain (+ exact receptive-field margins). Saves ~36% of conv matmul time
  for uniform lengths.
- Weight-resident conv loops: each conv weight chunk is DMA'd once per core
  (not once per sample): conv weight traffic 184MB -> 32MB (fp16).
- fp16 activations + weights through the conv chain (PE rate unchanged at
  1 cyc/col, but half the DMA/SBUF and 2x DVE pools).
- Model-parallel GEUs: the big 4096x4096 GatedEmbeddingUnits (text/video) and
  the 4096x1024 projection are output-sharded 8 ways. Per-core GEU weight
  streaming drops 8x (242us -> 30us of PE time). Requires 4 small collectives
  (AllGather t/v, AllGather x1, AllReduce sumsq, AllGather pooled-audio).
"""

import sys

for _p in ("/opt/trn_rl_repo", "/root/.axon_site/_ro/trn_rl_repo"):
    if _p not in sys.path:
        sys.path.append(_p)

from collections import deque
from contextlib import ExitStack

import numpy as np

import concourse.bass as bass
import concourse.mybir as mybir
import concourse.tile as tile
from concourse import bacc
from concourse.masks import make_identity

F32 = mybir.dt.float32
F32R = mybir.dt.float32r
F16 = mybir.dt.float16
I32 = mybir.dt.int32
AF = mybir.ActivationFunctionType
ALU = mybir.AluOpType

NEG = -3.0e38  # effectively -inf for max-pool padding
NS = 4          # samples (slots) per core
NC = 8          # cores
NB = NC * NS    # total batch
RG = [list(range(NC))]  # replica group


def derive_sizes(P):
    """Per-slot conv-chain extents from pooled lengths P (each mult of 4).

    Y* = conv-out cols computed at each layer, V* = exact input cols needed.
    All values are multiples of 8; pool(Y) produces exactly V_next cols.
    """
    S = {k: [] for k in ("Y5", "V5", "Y4", "V4", "Y3", "V3", "Y2", "V2",
                         "C4", "D4", "C5", "D5")}
    for Pa in P:
        assert 4 <= Pa <= 128 and Pa % 4 == 0
        y5 = min(256, 2 * Pa); v5 = min(256, y5 + 8)
        y4 = min(512, 2 * v5); v4 = min(512, y4 + 8)
        y3 = min(1024, 2 * v4); v3 = min(1024, y3 + 8)
        y2 = min(2048, 2 * v3); v2 = min(2048, ((y2 + 5 + 7) // 8) * 8)
        # conv4/5 compute extents padded to >=256 cols so f32r matmuls stay at
        # 1 cyc/row; cols beyond the exact prefix are garbage but finite
        # (pad regions of the input buffers are zeroed once).
        c5 = 256; d5 = 256
        c4 = max(256, y4); d4 = max(v4, c4)
        for k, v in (("Y5", y5), ("V5", v5), ("Y4", y4), ("V4", v4),
                     ("Y3", y3), ("V3", v3), ("Y2", y2), ("V2", v2),
                     ("C4", c4), ("D4", d4), ("C5", c5), ("D5", d5)):
            S[k].append(v)
    return S


def _segs(vals, halo):
    """offsets of per-slot segments [halo | data | halo]"""
    offs, o = [], 0
    for v in vals:
        offs.append(o)
        o += v + 2 * halo
    return offs, o


def tiles_of(Y, cap=512):
    """Balanced tile sizes (multiples of 8, each >=256 when Y >= 256)."""
    n = -(-Y // cap)
    base = (Y // n) // 8 * 8
    sizes = [base] * n
    rem, i = Y - base * n, 0
    while rem > 0:
        add = min(8, rem); sizes[i % n] += add; rem -= add; i += 1
    t0 = 0
    for w in sizes:
        yield t0, w
        t0 += w


def declare_io(nc, P, debug=False):
    S = derive_sizes(P)
    d = {"_S": S}

    def inp(name, shape, dt):
        d[name] = nc.dram_tensor(name, list(shape), dt, kind="ExternalInput")

    # per-core data
    inp("aT", (40, sum(S["V2"])), F32R)      # audio, per-slot valid prefixes
    inp("tT", (300, NS * 30), F16)           # text, (emb, sample*word)
    inp("vT", (4096, NS * 16), F16)          # video, (dim, sample*clip)
    inp("nf", (NS, 1), I32)                  # nframes//16 per slot (>=1)
    # conv weights (replicated); layout (coutp*cinp, cin128, tap*cout128).
    # f32r: self-loading matmuls (one PE instruction each, no Ldweights pair).
    inp("w1T", (40, 128), F32R)
    inp("b1", (128, 1), F32)
    inp("w2", (2, 128, 11 * 128), F32R)
    inp("b2", (128, 2), F32)
    inp("w3", (4 * 2, 128, 17 * 128), F32R)
    inp("b3", (128, 4), F32)
    inp("w4", (4 * 4, 128, 17 * 128), F32R)
    inp("b4", (128, 4), F32)
    inp("w5", (8 * 4, 128, 17 * 128), F32R)
    inp("b5", (128, 8), F32)
    # text branch (replicated)
    inp("tpT", (300, 4096), F16)
    inp("tpb", (128, 32), F32)
    # model-parallel GEU weight slices (per-core! each core gets its own
    # 512-wide output slice), layout (128, nk*512) k-major
    for nm in ("gutf", "gutc", "guvf", "guvc"):
        inp(nm + "T", (128, 32 * 512), F16)
        inp(nm + "b", (1, 512), F16)
    inp("projT", (128, 8 * 512), F16)
    inp("projb", (1, 512), F16)
    # audio GEU (1024), replicated
    for nm in ("guaf", "guac"):
        inp(nm + "T", (128, 8 * 1024), F16)
        inp(nm + "b", (1, 1024), F16)

    d["out"] = nc.dram_tensor("out", [3, NB, 512], F32, kind="ExternalOutput")
    if debug:
        _, x2tot = _segs(S["V2"], 5)
        _, x3tot = _segs(S["V3"], 8)
        _, x4tot = _segs(S["D4"], 8)
        _, x5tot = _segs(S["D5"], 8)
        d["dbg_x2"] = nc.dram_tensor("dbg_x2", [128, x2tot], F32, kind="ExternalOutput")
        d["dbg_x3"] = nc.dram_tensor("dbg_x3", [2 * 128, x3tot], F32, kind="ExternalOutput")
        d["dbg_x4"] = nc.dram_tensor("dbg_x4", [4 * 128, x4tot], F32, kind="ExternalOutput")
        d["dbg_x5"] = nc.dram_tensor("dbg_x5", [4 * 128, x5tot], F32, kind="ExternalOutput")
        d["dbg_a"] = nc.dram_tensor("dbg_a", [8 * 128, sum(P)], F32, kind="ExternalOutput")
        d["dbg_pool"] = nc.dram_tensor("dbg_pool", [128, 8 * NS], F32, kind="ExternalOutput")
        d["dbg_t"] = nc.dram_tensor("dbg_t", [128, 128], F32, kind="ExternalOutput")
        d["dbg_tall"] = nc.dram_tensor("dbg_tall", [128, 1024], F32, kind="ExternalOutput")
        d["dbg_x1g"] = nc.dram_tensor("dbg_x1g", [32, 512], F32, kind="ExternalOutput")
        d["dbg_x1gT"] = nc.dram_tensor("dbg_x1gT", [128, 1024], F32, kind="ExternalOutput")
    return d


def emit(ctx: ExitStack, tc: tile.TileContext, d, P, debug=False):
    nc = tc.nc
    S = d["_S"]
    Y5, V5, Y4, V4 = S["Y5"], S["V5"], S["Y4"], S["V4"]
    Y3, V3, Y2, V2 = S["Y3"], S["V3"], S["Y2"], S["V2"]
    C4, D4, C5, D5 = S["C4"], S["D4"], S["C5"], S["D5"]
    seg2, x2tot = _segs(V2, 5)
    seg3, x3tot = _segs(V3, 8)
    seg4, x4tot = _segs(D4, 8)
    seg5, x5tot = _segs(D5, 8)
    sega = [sum(P[:s]) for s in range(NS)]
    atot = sum(P)
    au_off = [sum(V2[:s]) for s in range(NS)]

    # ---------------- pools ----------------
    consts = ctx.enter_context(tc.tile_pool(name="consts", bufs=1))
    acts = ctx.enter_context(tc.tile_pool(name="acts", bufs=1))
    wconv = ctx.enter_context(tc.tile_pool(name="wconv", bufs=2))
    ystream = ctx.enter_context(tc.tile_pool(name="ystream", bufs=2))
    geu_sb = ctx.enter_context(tc.tile_pool(name="geu_sb", bufs=1))
    gstream = ctx.enter_context(tc.tile_pool(name="gstream", bufs=3))
    small = ctx.enter_context(tc.tile_pool(name="small", bufs=2))
    dram = ctx.enter_context(tc.tile_pool(name="dram", bufs=1, space="DRAM"))

    psum_conv = ctx.enter_context(tc.tile_pool(name="psum_conv", bufs=2, space="PSUM"))
    psum_geu = ctx.enter_context(tc.tile_pool(name="psum_geu", bufs=1, space="PSUM"))
    psum_tp = ctx.enter_context(tc.tile_pool(name="psum_tp", bufs=1, space="PSUM"))

    # ---------------- collective bounce buffers ----------------
    ag1_in = dram.tile([128, 256], F16)               # [tT16 | vT16]
    ag1_out = dram.tile([NC, 128, 2, 32, NS], F16)    # (c, i, b, k, j)
    ag2_in = dram.tile([128, 256], F16)               # [gut x1T | guv x1T]
    ag2_out = dram.tile([NC, 128, 2, 4, 32], F16)     # (c, i, b, kt, p)
    ar3_in = dram.tile([32, 2], F32)
    ar3_out = dram.tile([32, 2], F32)
    ag4_in = dram.tile([NS, 1024], F32)
    ag4_out = dram.tile([32, 1024], F32)

    # ---------------- constants ----------------
    ident = consts.tile([32, 32], F32)
    make_identity(nc, ident[:])
    ones_f = consts.tile([128, 1], F32)
    nc.vector.memset(ones_f[:], 1.0)
    ones_r = consts.tile([128, 1], F32R)
    nc.vector.tensor_copy(ones_r[:], ones_f[:])
    ones_row_f = consts.tile([1, 32], F32)
    nc.vector.memset(ones_row_f[:], 1.0)
    ones_row_h = consts.tile([1, 32], F16)
    nc.vector.tensor_copy(ones_row_h[:], ones_row_f[:])

    b1t = consts.tile([128, 1], F32); nc.sync.dma_start(b1t[:], d["b1"][:, :])
    b2t = consts.tile([128, 2], F32); nc.sync.dma_start(b2t[:], d["b2"][:, :])
    b3t = consts.tile([128, 4], F32); nc.sync.dma_start(b3t[:], d["b3"][:, :])
    b4t = consts.tile([128, 4], F32); nc.sync.dma_start(b4t[:], d["b4"][:, :])
    b5t = consts.tile([128, 8], F32); nc.sync.dma_start(b5t[:], d["b5"][:, :])
    tpbt = consts.tile([128, 32], F32); nc.sync.dma_start(tpbt[:], d["tpb"][:, :])

    # ---------------- mask for audio masked-mean ----------------
    nfi = small.tile([NS, 1], I32)
    nc.sync.dma_start(nfi[:], d["nf"][:, :])
    nff = small.tile([NS, 1], F32)
    nc.vector.tensor_copy(nff[:], nfi[:])
    rnf = small.tile([NS, 1], F32)
    nc.vector.reciprocal(rnf[:], nff[:])
    iot = small.tile([NS, 128], I32)
    nc.gpsimd.iota(iot[:], pattern=[[1, 128]], base=0, channel_multiplier=0)
    iotf = small.tile([NS, 128], F32)
    nc.vector.tensor_copy(iotf[:], iot[:])
    mrow = small.tile([NS, 128], F32)
    nc.vector.tensor_scalar(mrow[:], iotf[:], nff[:], None, ALU.is_lt)
    mrow2 = small.tile([NS, 128], F32)
    nc.vector.tensor_scalar_mul(mrow2[:], mrow[:], rnf[:])
    mbs = []
    for s in range(NS):
        stage = small.tile([1, 128], F32, name=f"mstage{s}", tag="mstage")
        nc.sync.dma_start(stage[:], mrow2[s:s + 1, :])
        mb = consts.tile([128, 128], F32, name=f"mb{s}")
        nc.gpsimd.partition_broadcast(mb[:], stage[:])
        mbs.append(mb)

    # ---------------- persistent activation buffers (f32r, aliased) -------
    # lifetimes: X2 [conv1,conv2], X3 [conv2,conv3], X4 [conv3,conv4],
    # X5 [conv4,conv5] -> X4 shares slot1 with X2; X5 shares slot2 with X3.
    slot1_w = max(x2tot, 4 * x4tot)
    slot2_w = max(2 * x3tot, 4 * x5tot)
    X2 = acts.tile([128, slot1_w], F32R, tag="slot1")
    X3 = acts.tile([128, slot2_w], F32R, tag="slot2")
    A = acts.tile([128, 8 * atot], F32)
    X4 = X5 = None  # allocated later (alias slots)

    def x3c(c): return X3[:, c * x3tot:(c + 1) * x3tot]
    def x4c(c): return X4[:, c * x4tot:(c + 1) * x4tot]
    def x5c(c): return X5[:, c * x5tot:(c + 1) * x5tot]
    def ac(c): return A[:, c * atot:(c + 1) * atot]

    def zero_halos(buf, segs, vals, halo, nch, tot, dvals=None):
        """halo zeroing + (for padded layers) finite-fill of pad cols."""
        for ch in range(nch):
            for s in range(NS):
                o = ch * tot + segs[s]
                nc.vector.memset(buf[:, o:o + halo].bitcast(F32), 0.0)
                dv = vals[s] if dvals is None else dvals[s]
                nc.vector.memset(buf[:, o + halo + vals[s]:o + 2 * halo + dv].bitcast(F32), 0.0)

    zero_halos(X2, seg2, V2, 5, 1, x2tot)
    zero_halos(X3, seg3, V3, 8, 2, x3tot)

    # ---------------- conv1: (40 -> 128), k=1, relu ----------------
    w1 = consts.tile([40, 128], F32R)
    nc.sync.dma_start(w1[:], d["w1T"][:, :])
    for s in range(NS):
        for t0, w in tiles_of(V2[s]):
            ain = ystream.tile([40, 512], F32R, tag="ain")
            nc.sync.dma_start(ain[:, 0:w], d["aT"][:, au_off[s] + t0: au_off[s] + t0 + w])
            ps = psum_conv.tile([128, 512], F32, tag="cps")
            nc.tensor.matmul(ps[:, 0:w], w1[:], ain[:, 0:w], start=True, stop=True)
            nc.scalar.activation(X2[:, seg2[s] + 5 + t0: seg2[s] + 5 + t0 + w],
                                 ps[:, 0:w], AF.Relu, bias=b1t[:, 0:1])

    # ---------------- text branch (local samples) -> tT16 ----------------
    tT16 = geu_sb.tile([128, 128], F16)
    kszs = [128, 128, 44]
    tTin = []
    for ki, kp in enumerate(kszs):
        t_ = consts.tile([kp, NS * 30], F16, name=f"tTin{ki}")
        nc.sync.dma_start(t_[:], d["tT"][ki * 128: ki * 128 + kp, :])
        tTin.append(t_)
    for o in range(32):
        ps = psum_conv.tile([128, NS * 30], F32, tag="cps")
        for ki, kp in enumerate(kszs):
            wt = ystream.tile([128, 128], F16, tag="tpw", bufs=4)
            nc.sync.dma_start(wt[0:kp, :], d["tpT"][ki * 128: ki * 128 + kp,
                                                    o * 128:(o + 1) * 128])
            nc.tensor.matmul(ps[:], wt[0:kp, :], tTin[ki][:],
                             start=(ki == 0), stop=(ki == 2))
        tw = ystream.tile([128, NS * 30], F32, tag="tw")
        nc.scalar.activation(tw[:], ps[:], AF.Relu, bias=tpbt[:, o:o + 1])
        tmax = ystream.tile([128, NS], F32, tag="tmax")
        nc.vector.reduce_max(tmax[:], tw[:].rearrange("p (s w) -> p s w", s=NS),
                             axis=mybir.AxisListType.X, opt_input=False)
        nc.vector.tensor_copy(tT16[:, o * NS:(o + 1) * NS], tmax[:])

    nc.gpsimd.dma_start(ag1_in[:, 0:128], tT16[:])
    if debug:
        nc.sync.dma_start(d["dbg_t"][:, :], tT16[:].bitcast(F16).rearrange("p n -> p n"))

    # ---------------- video branch (local samples) -> vT16 ----------------
    vT16 = geu_sb.tile([128, 128], F16)
    vchbuf = geu_sb.tile([128, 128], F32)
    ssv_ps = psum_tp.tile([1, NS], F32, tag="tpp")
    for c in range(32):
        vin = ystream.tile([128, NS * 16], F16, tag="vin")
        nc.sync.dma_start(vin[:], d["vT"][c * 128:(c + 1) * 128, :])
        nc.vector.reduce_max(vchbuf[:, c * NS:(c + 1) * NS],
                             vin[:].rearrange("p (s k) -> p s k", s=NS),
                             axis=mybir.AxisListType.X, opt_input=False)
        vsq = ystream.tile([128, NS], F32R, tag="vsq")
        nc.vector.tensor_tensor(vsq[:], vchbuf[:, c * NS:(c + 1) * NS],
                                vchbuf[:, c * NS:(c + 1) * NS], ALU.mult)
        nc.tensor.matmul(ssv_ps[:], ones_r[:], vsq[:], start=(c == 0), stop=(c == 31))
    ssv = small.tile([1, NS], F32)
    nc.vector.tensor_scalar_max(ssv[:], ssv_ps[:], 1e-24)
    ssq = small.tile([1, NS], F32)
    nc.scalar.activation(ssq[:], ssv[:], AF.Sqrt)
    ssr = small.tile([1, NS], F32)
    nc.vector.reciprocal(ssr[:], ssq[:])
    invb = consts.tile([128, NS], F32)
    nc.gpsimd.partition_broadcast(invb[:], ssr[:])
    for c in range(32):
        nc.vector.tensor_tensor(vT16[:, c * NS:(c + 1) * NS],
                                vchbuf[:, c * NS:(c + 1) * NS], invb[:], ALU.mult)

    nc.gpsimd.dma_start(ag1_in[:, 128:256], vT16[:])

    # ---------------- AG1: gather t/v chunks for all 32 samples ----------
    nc.gpsimd.collective_compute(
        "AllGather", ALU.bypass, replica_groups=RG,
        ins=[ag1_in[:].opt()], outs=[ag1_out[:].opt()])
    # readback lands core-major (8 contiguous [128, 128] blocks -> fast DMA),
    # then cheap DVE strided copies shuffle into k-major chunks [128, 32] so
    # the f-linear lhsT has a single free dim.
    tT_all = geu_sb.tile([128, 32 * 32], F16)
    vT_all = geu_sb.tile([128, 32 * 32], F16)
    for b, dst in ((0, tT_all), (1, vT_all)):
        cm = ystream.tile([128, 32 * 32], F16, tag="geu_tmp", name=f"cm{b}")
        for c in range(NC):
            src = ag1_out[c, :, b, :, :]  # (i, k, j) contiguous 128
            # gpsimd queue: lands right behind the collective, ahead of the
            # long sync/scalar DMA backlogs
            nc.gpsimd.dma_start(cm[:, c * 128:(c + 1) * 128], src)
        dv = dst[:].rearrange("p (k c j) -> p k c j", k=32, c=NC)
        for c in range(NC):
            nc.vector.tensor_copy(
                dv[:, :, c, :],
                cm[:, c * 128:(c + 1) * 128].rearrange("p (k j) -> p k j", k=32))

    def chunk_of(dst):
        return lambda kk: dst[:, kk * 32:(kk + 1) * 32]
    if debug:
        nc.sync.dma_start(d["dbg_tall"][:, :], tT_all[:].bitcast(F16).rearrange("p n -> p n"))

    # ---------------- model-parallel GEU machinery -----------------------
    # out_slice[32, 512] = sum_k xT[k][128,32].T @ W[k][128,512]  (+ bias row)
    KI = 4

    def mp_linear_items(wkey, xT_fn, nk, epi):
        st = {}
        n_items = nk // KI

        def dma_fn(i):
            if i == 0:
                st["ps"] = psum_geu.tile([32, 512], F32, tag="gps", name="gps")
                brow = small.tile([1, 512], F16, tag="brow", name="brow")
                nc.scalar.dma_start(brow[:], d[wkey + "b"][0:1, :])
                st["brow"] = brow
            wt = gstream.tile([128, KI * 512], F16, tag="gw", name="gw")
            nc.scalar.dma_start(wt[:], d[wkey + "T"][:, i * KI * 512:(i + 1) * KI * 512])
            st[i] = wt

        def mm_fn(i):
            wt = st.pop(i)
            ps = st["ps"]
            for k in range(KI):
                kk = i * KI + k
                nc.tensor.matmul(ps[:], xT_fn(kk),
                                 wt[:, k * 512:(k + 1) * 512],
                                 start=(kk == 0), stop=False)
            if i == n_items - 1:
                brow = st.pop("brow")
                nc.tensor.matmul(ps[:], ones_row_h[:], brow[:], start=False, stop=True)
                epi(st.pop("ps"))

        for i in range(n_items):
            yield (lambda i=i: dma_fn(i)), (lambda i=i: mm_fn(i))

    class MPGeu:
        def __init__(self, name, xT_fn, fkey, ckey, out_row, ag2_col):
            self.name, self.xT_fn = name, xT_fn
            self.fkey, self.ckey = fkey, ckey
            self.out_row, self.ag2_col = out_row, ag2_col
            self.x1 = geu_sb.tile([32, 512], F32, name=f"{name}_x1")
            self.x2 = geu_sb.tile([32, 512], F32, name=f"{name}_x2")
            self.x1T_loc = geu_sb.tile([128, 128], F16, name=f"{name}_x1Tl")
            self.xcT = geu_sb.tile([128, 32 * 32], F16, name=f"{name}_xcT")

        def f_items(self):
            yield from mp_linear_items(self.fkey, self.xT_fn, 32, self.f_epi)
            yield (None, self.transpose_x1)

        def f_epi(self, ps):
            nc.scalar.copy(self.x1[:], ps[:])

        def transpose_x1(self):
            if debug and self.name == "gut":
                nc.sync.dma_start(d["dbg_x1g"][:, :], self.x1[:, :])
            for k in range(4):
                tp = psum_tp.tile([128, 32], F32, tag="tpp")
                nc.tensor.transpose(tp[:], self.x1[:, k * 128:(k + 1) * 128],
                                    ident[0:32, 0:32])
                nc.scalar.copy(self.x1T_loc[:, k * 32:(k + 1) * 32], tp[:])
            nc.gpsimd.dma_start(ag2_in[:, self.ag2_col:self.ag2_col + 128],
                                self.x1T_loc[:])

        def c_items(self, ssb2):
            yield from mp_linear_items(
                self.ckey, lambda kk: self.xcT[:, kk * 32:(kk + 1) * 32], 32,
                lambda ps: self.c_epi(ps, ssb2))

        def c_epi(self, ps, ssb2):
            sg = ystream.tile([32, 512], F32, tag="geu_tmp", name="sg")
            nc.scalar.activation(sg[:], ps[:], AF.Sigmoid)
            nc.vector.tensor_tensor(self.x2[:], self.x1[:], sg[:], ALU.mult)
            sq = ystream.tile([32, 512], F32, tag="geu_tmp", name="sq")
            nc.scalar.activation(sq[:], self.x2[:], AF.Square,
                                 accum_out=ssb2[:, self.out_row:self.out_row + 1])

    gut = MPGeu("gut", chunk_of(tT_all), "gutf", "gutc", 0, 0)
    guv = MPGeu("guv", chunk_of(vT_all), "guvf", "guvc", 1, 128)
    ssb2 = small.tile([32, 2], F32, name="ssb2")

    def ag2_and_readback():
        nc.gpsimd.collective_compute(
            "AllGather", ALU.bypass, replica_groups=RG,
            ins=[ag2_in[:].opt()], outs=[ag2_out[:].opt()])
        for b, g in ((0, gut), (1, guv)):
            for c in range(NC):
                src = ag2_out[c, :, b, :, :]  # (i, kt, p) -> contiguous 128
                nc.gpsimd.dma_start(g.xcT[:, c * 128:(c + 1) * 128], src)
        if debug:
            nc.sync.dma_start(d["dbg_x1gT"][:, :],
                              gut.xcT[:].bitcast(F16).rearrange("p n -> p n"))

    def ar3_and_out():
        nc.gpsimd.dma_start(ar3_in[:], ssb2[:])
        nc.gpsimd.collective_compute(
            "AllReduce", ALU.add, replica_groups=RG,
            ins=[ar3_in[:].opt()], outs=[ar3_out[:].opt()])
        ssg = small.tile([32, 2], F32, name="ssg")
        nc.gpsimd.dma_start(ssg[:], ar3_out[:])
        ssm = small.tile([32, 2], F32, name="ssm")
        nc.vector.tensor_scalar_max(ssm[:], ssg[:], 1e-24)
        ssq_ = small.tile([32, 2], F32, name="ssq_")
        nc.scalar.activation(ssq_[:], ssm[:], AF.Sqrt)
        inv2 = small.tile([32, 2], F32, name="inv2")
        nc.vector.reciprocal(inv2[:], ssq_[:])
        for b, g in ((0, gut), (1, guv)):
            o_sb = ystream.tile([32, 512], F32, tag="geu_tmp", name="o_sb")
            nc.vector.tensor_scalar_mul(o_sb[:], g.x2[:], inv2[:, b:b + 1])
            nc.sync.dma_start(d["out"][b, :, :], o_sb[:])

    _items = deque()
    _items.append((None, lambda: None))
    _items.append((None, lambda: None))
    _items.extend(gut.f_items())
    _items.extend(guv.f_items())
    _items.append((None, ag2_and_readback))
    for _ in range(9):
        _items.append((None, lambda: None))
    _items.extend(gut.c_items(ssb2))
    _items.extend(guv.c_items(ssb2))
    _items.append((None, ar3_and_out))
    _pending = deque()

    def step():
        if _items:
            dma_fn, mm_fn = _items.popleft()
            if dma_fn is not None:
                dma_fn()
            _pending.append(mm_fn)
            if len(_pending) > 1:
                _pending.popleft()()
        elif _pending:
            _pending.popleft()()

    def flush():
        while _items or _pending:
            step()

    # ---------------- shared conv helpers ----------------
    def maxpool_into(dst_ap, ybuf, width, tag, dt_):
        """dst[j] = max(y[2j-1],y[2j],y[2j+1]); ybuf [128, 2*width+2] padded."""
        even = ybuf[:, 0:2 * width].rearrange("p (j two) -> p j two", two=2)
        odd2 = ybuf[:, 2:2 * width + 2].rearrange("p (j two) -> p j two", two=2)
        m1 = ystream.tile([128, width], dt_, tag=tag)
        nc.vector.tensor_tensor(m1[:], even[:, :, 0], even[:, :, 1], ALU.max)
        nc.vector.tensor_tensor(dst_ap, m1[:], odd2[:, :, 0], ALU.max)

    def conv_layer(wkey, bt, n_co, n_ci, taps, xin_c, seg_in, Ys, out_fn,
                   ytag, mtag, ydt):
        """conv2/conv3 path: all ci chunks resident, per-tile psum chains."""
        halo_w = taps * 128
        for co in range(n_co):
            wts = []
            for ci in range(n_ci):
                wt = wconv.tile([128, halo_w], F32R, tag=f"wc{ci}", name=f"wc{ci}")
                nc.sync.dma_start(wt[:], d[wkey][co * n_ci + ci, :, :])
                wts.append(wt)
            for s in range(NS):
                yb = ystream.tile([128, Ys[s] + 2], ydt, tag=ytag, name=ytag)
                nc.vector.memset(yb[:, 0:1], NEG)
                nc.vector.memset(yb[:, Ys[s] + 1:Ys[s] + 2], NEG)
                for t0, w in tiles_of(Ys[s]):
                    ps = psum_conv.tile([128, 512], F32, tag="cps", name="cps")
                    for ci in range(n_ci):
                        for tap in range(taps):
                            nc.tensor.matmul(
                                ps[:, 0:w], wts[ci][:, tap * 128:(tap + 1) * 128],
                                xin_c(ci)[:, seg_in[s] + t0 + tap: seg_in[s] + t0 + tap + w],
                                start=(ci == 0 and tap == 0),
                                stop=(ci == n_ci - 1 and tap == taps - 1))
                    nc.scalar.activation(yb[:, 1 + t0: 1 + t0 + w], ps[:, 0:w],
                                         AF.Relu, bias=bt[:, co:co + 1])
                    step()
                out_fn(co, s, yb)

    def conv_layer_pass(wkey, bt, n_co, taps, xin_c, seg_in, Cs, out_fn, ytag,
                        ydt=F32):
        """conv4/conv5 path (n_ci=4): two ci-pair passes, per-sample psum tiles
        kept alive across both passes (only 2 weight tags resident)."""
        halo_w = taps * 128
        for co in range(n_co):
            pss = [psum_conv.tile([128, Cs[s]], F32, tag=f"cp{s}", bufs=1,
                                  name=f"cp{s}") for s in range(NS)]
            for ph in range(2):
                wts = []
                for q in range(2):
                    wt = wconv.tile([128, halo_w], F32R, tag=f"wc{q}", name=f"wc{q}")
                    nc.sync.dma_start(wt[:], d[wkey][co * 4 + ph * 2 + q, :, :])
                    wts.append(wt)
                for s in range(NS):
                    for q in range(2):
                        ci = ph * 2 + q
                        for tap in range(taps):
                            nc.tensor.matmul(
                                pss[s][:], wts[q][:, tap * 128:(tap + 1) * 128],
                                xin_c(ci)[:, seg_in[s] + tap: seg_in[s] + tap + Cs[s]],
                                start=(ci == 0 and tap == 0),
                                stop=(ci == 3 and tap == taps - 1))
                    step()
            for s in range(NS):
                yb = ystream.tile([128, Cs[s] + 2], ydt, tag=ytag, name=ytag)
                nc.vector.memset(yb[:, 0:1], NEG)
                nc.vector.memset(yb[:, Cs[s] + 1:Cs[s] + 2], NEG)
                nc.scalar.activation(yb[:, 1: 1 + Cs[s]], pss[s][:],
                                     AF.Relu, bias=bt[:, co:co + 1])
                step()
                out_fn(co, s, yb)

    # ---------------- conv2: 128 -> 256, k=11 ----------------
    def out2(co, s, yb):
        maxpool_into(x3c(co)[:, seg3[s] + 8: seg3[s] + 8 + Y2[s] // 2],
                     yb, Y2[s] // 2, "mp2", F16)

    conv_layer("w2", b2t, 2, 1, 11, lambda ci: X2, seg2, Y2, out2, "y2", "mp2", F16)

    # ---------------- conv3: 256 -> 512, k=17 ----------------
    X4 = acts.tile([128, slot1_w], F32R, tag="slot1", name="X4")
    zero_halos(X4, seg4, V4, 8, 4, x4tot, dvals=D4)

    def out3(co, s, yb):
        maxpool_into(x4c(co)[:, seg4[s] + 8: seg4[s] + 8 + Y3[s] // 2],
                     yb, Y3[s] // 2, "mp3", F16)

    conv_layer("w3", b3t, 4, 2, 17, x3c, seg3, Y3, out3, "y3", "mp3", F16)

    # ---------------- conv4: 512 -> 512, k=17 (padded to >=256) -----------
    X5 = acts.tile([128, slot2_w], F32R, tag="slot2", name="X5")
    zero_halos(X5, seg5, [C4[s] // 2 for s in range(NS)], 8, 4, x5tot, dvals=D5)

    def out4(co, s, yb):
        maxpool_into(x5c(co)[:, seg5[s] + 8: seg5[s] + 8 + C4[s] // 2],
                     yb, C4[s] // 2, "mp4", F16)

    conv_layer_pass("w4", b4t, 4, 17, x4c, seg4, C4, out4, "y4", ydt=F16)

    # ---------------- conv5: 512 -> 1024, k=17, + masked mean -------------
    xTg32 = geu_sb.tile([128, 8 * NS], F32)

    def out5(co, s, yb):
        maxpool_into(ac(co)[:, sega[s]: sega[s] + P[s]], yb, P[s], "mp5", F32)
        scr = ystream.tile([128, 128], F32, tag="mmean")
        nc.vector.scalar_tensor_tensor(
            scr[:, 0:P[s]], ac(co)[:, sega[s]: sega[s] + P[s]], 1.0,
            mbs[s][:, 0:P[s]], ALU.mult, ALU.mult,
            accum_out=xTg32[:, co * NS + s: co * NS + s + 1])

    conv_layer_pass("w5", b5t, 8, 17, x5c, seg5, C5, out5, "y5")

    if debug:
        nc.sync.dma_start(d["dbg_pool"][:, :], xTg32[:, :])
        for (nm, buf, nch, tot) in (("dbg_x2", X2, 1, x2tot), ("dbg_x3", X3, 2, x3tot),
                                    ("dbg_x4", X4, 4, x4tot), ("dbg_x5", X5, 4, x5tot)):
            for c in range(nch):
                f32v = ystream.tile([128, tot], F32, tag="dbgcp", bufs=1, name=f"dbg{nm}{c}")
                nc.vector.tensor_copy(f32v[:], buf[:, c * tot:(c + 1) * tot])
                nc.sync.dma_start(d[nm][c * 128:(c + 1) * 128, :], f32v[:])
        for c in range(8):
            f32v = ystream.tile([128, atot], F32, tag="dbgcp", bufs=1, name=f"dbgA{c}")
            nc.vector.tensor_copy(f32v[:], ac(c))
            nc.sync.dma_start(d["dbg_a"][c * 128:(c + 1) * 128, :], f32v[:])

    flush()

    # ---------------- audio GEU (1024, local samples) --------------------
    xTgh = geu_sb.tile([128, 8 * NS], F16)
    nc.vector.tensor_copy(xTgh[:], xTg32[:])
    ones_row_h4 = consts.tile([1, NS], F16)
    nc.vector.memset(ones_row_h4[:], 1.0)

    def gua_linear(wkey, xT, out_sb):
        pss = [psum_geu.tile([NS, 512], F32, tag="gps", name="agps0"),
               psum_tp.tile([NS, 512], F32, tag="tpp", name="agps1")]
        browa = small.tile([1, 1024], F16, tag="browa", name="browa")
        nc.scalar.dma_start(browa[:], d[wkey + "b"][0:1, :])
        for i in range(4):
            wt = gstream.tile([128, KI * 512], F16, tag="gw", name="gwa")
            nc.scalar.dma_start(wt[:], d[wkey + "T"][:, i * 2048:(i + 1) * 2048])
            for k in range(2):
                kk = i * 2 + k
                for j in range(2):
                    nc.tensor.matmul(pss[j][:], xT[:, kk * NS:(kk + 1) * NS],
                                     wt[:, k * 1024 + j * 512: k * 1024 + (j + 1) * 512],
                                     start=(kk == 0), stop=False)
        for j in range(2):
            nc.tensor.matmul(pss[j][:], ones_row_h4[:],
                             browa[:, j * 512:(j + 1) * 512], start=False, stop=True)
        for j in range(2):
            nc.scalar.copy(out_sb[:, j * 512:(j + 1) * 512], pss[j][:])

    x1a = geu_sb.tile([NS, 1024], F32, name="x1a")
    gua_linear("guaf", xTgh, x1a)
    x1aT = geu_sb.tile([128, 8 * NS], F16, name="x1aT")
    for k in range(8):
        tp = psum_tp.tile([128, NS], F32, tag="tpp")
        nc.tensor.transpose(tp[:], x1a[:, k * 128:(k + 1) * 128], ident[0:NS, 0:NS])
        nc.scalar.copy(x1aT[:, k * NS:(k + 1) * NS], tp[:])
    g1a = ystream.tile([NS, 1024], F32, tag="gua_tmp", name="g1a")
    gua_linear("guac", x1aT, g1a)
    sga = ystream.tile([NS, 1024], F32, tag="gua_tmp", name="sga")
    nc.scalar.activation(sga[:], g1a[:], AF.Sigmoid)
    x2a = geu_sb.tile([NS, 1024], F32, name="x2a")
    ssa = small.tile([NS, 1], F32, name="ssa")
    nc.vector.tensor_tensor(x2a[:], x1a[:], sga[:], ALU.mult)
    sqa = ystream.tile([NS, 1024], F32, tag="gua_tmp", name="sqa")
    nc.scalar.activation(sqa[:], x2a[:], AF.Square, accum_out=ssa[:, 0:1])
    ssam = small.tile([NS, 1], F32, name="ssam")
    nc.vector.tensor_scalar_max(ssam[:], ssa[:], 1e-24)
    ssaq = small.tile([NS, 1], F32, name="ssaq")
    nc.scalar.activation(ssaq[:], ssam[:], AF.Sqrt)
    inva = small.tile([NS, 1], F32, name="inva")
    nc.vector.reciprocal(inva[:], ssaq[:])
    ga = x1a  # x1a is dead after x2a; reuse its space for the normalized output
    nc.vector.tensor_scalar_mul(ga[:], x2a[:], inva[:, 0:1])

    # ---------------- AG4: gather pooled-audio GEU outputs ----------------
    nc.gpsimd.dma_start(ag4_in[:], ga[:])
    nc.gpsimd.collective_compute(
        "AllGather", ALU.bypass, replica_groups=RG,
        ins=[ag4_in[:].opt()], outs=[ag4_out[:].opt()])
    ga_all = geu_sb.tile([32, 1024], F32, name="ga_all")
    nc.sync.dma_start(ga_all[:], ag4_out[:])
    gaT = geu_sb.tile([128, 8 * 32], F16, name="gaT")
    for k in range(8):
        tp = psum_tp.tile([128, 32], F32, tag="tpp")
        nc.tensor.transpose(tp[:], ga_all[:, k * 128:(k + 1) * 128], ident[0:32, 0:32])
        nc.scalar.copy(gaT[:, k * 32:(k + 1) * 32], tp[:])

    # ---------------- projection slice -> out[2] --------------------------
    psp = psum_geu.tile([32, 512], F32, tag="gps", name="pgps")
    browp = small.tile([1, 512], F16, tag="brow", name="browp")
    nc.scalar.dma_start(browp[:], d["projb"][0:1, :])
    for i in range(2):
        wt = gstream.tile([128, 4 * 512], F16, tag="gw", name="gwp")
        nc.scalar.dma_start(wt[:], d["projT"][:, i * 2048:(i + 1) * 2048])
        for k in range(4):
            kk = i * 4 + k
            nc.tensor.matmul(psp[:], gaT[:, kk * 32:(kk + 1) * 32],
                             wt[:, k * 512:(k + 1) * 512],
                             start=(kk == 0), stop=False)
    nc.tensor.matmul(psp[:], ones_row_h[:], browp[:], start=False, stop=True)
    ot_sb = ystream.tile([32, 512], F32, tag="geu_tmp", name="ot_sb")
    nc.scalar.copy(ot_sb[:], psp[:])
    nc.sync.dma_start(d["out"][2, :, :], ot_sb[:])


def build(P, debug=False):
    nc = bacc.Bacc()
    d = declare_io(nc, P, debug=debug)
    with tile.TileContext(nc) as tc:
        with ExitStack() as ctx:
            emit(ctx, tc, d, P, debug=debug)
    nc.compile()
    return nc


# ---------------------------------------------------------------------------
# host-side planning + data prep
# ---------------------------------------------------------------------------
def plan_from_inputs(inputs):
    """sample -> (core, slot) assignment and compiled slot lengths P."""
    nfr = np.asarray(inputs["audio_STFT_nframes"]).astype(np.int64)
    nf = np.maximum(1, nfr // 16)
    order = np.argsort(-nf, kind="stable")
    P = []
    for j in range(NS):
        Pa = int(nf[order[j * NC:(j + 1) * NC]].max())
        P.append(min(128, ((Pa + 3) // 4) * 4))
    return order, tuple(P)


def prep_shared(inp):
    """Replicated weights, host-transposed/cast."""
    f32, f16 = np.float32, np.float16
    w = {}
    bn_scale = (np.asarray(inp["bn_g"])[0] /
                np.sqrt(np.float32(1.0) + np.float32(1e-5))).astype(f32)
    c1 = np.asarray(inp["c1w"])[:, 0, :, 0].astype(f32)   # (128, 40)
    w["w1T"] = np.ascontiguousarray((c1 * bn_scale).T)
    w["b1"] = np.ascontiguousarray(
        (np.asarray(inp["c1b"]) + np.asarray(inp["bn_b"])[0] * c1.sum(1)).astype(f32)[:, None])

    def conv_w(cw, coutp, cinp, taps):
        cw = np.asarray(cw)
        ci = cw.shape[1]
        cin = ci // cinp
        a = cw[:, :, 0, :].astype(f32)                    # (Cout, Cin, taps)
        a = a.reshape(coutp, 128, cinp, cin, taps)
        a = a.transpose(0, 2, 3, 4, 1)                    # coutp, cinp, cin, tap, cout
        return np.ascontiguousarray(a.reshape(coutp * cinp, cin, taps * 128))

    def bias_t(b, coutp):
        return np.ascontiguousarray(np.asarray(b).astype(f32).reshape(coutp, 128).T)

    w["w2"] = conv_w(inp["c2w"], 2, 1, 11); w["b2"] = bias_t(inp["c2b"], 2)
    w["w3"] = conv_w(inp["c3w"], 4, 2, 17); w["b3"] = bias_t(inp["c3b"], 4)
    w["w4"] = conv_w(inp["c4w"], 4, 4, 17); w["b4"] = bias_t(inp["c4b"], 4)
    w["w5"] = conv_w(inp["c5w"], 8, 4, 17); w["b5"] = bias_t(inp["c5b"], 8)

    w["tpT"] = np.ascontiguousarray(np.asarray(inp["tp_w"]).astype(f32).T.astype(f16))
    w["tpb"] = np.ascontiguousarray(np.asarray(inp["tp_b"]).astype(f32).reshape(32, 128).T)

    for nm, src in (("guaf", "gua_fw"), ("guac", "gua_cw")):
        wT = np.asarray(inp[src]).astype(f32).T.astype(f16)   # (1024, 1024)
        a = wT.reshape(8, 128, 1024).transpose(1, 0, 2)
        w[nm + "T"] = np.ascontiguousarray(a.reshape(128, 8 * 1024))
        w[nm + "b"] = np.ascontiguousarray(
            np.asarray(inp[src.replace("w", "b")]).astype(f16)[None, :])
    return w


def prep_core_inputs(inp, w, order, P, core):
    """Per-core input map: local samples + this core's GEU weight slices."""
    f16 = np.float16
    S = derive_sizes(P)
    m = dict(w)
    samples = [int(order[NC * j + core]) for j in range(NS)]

    audio = np.asarray(inp["audio"]).astype(np.float32)
    m["aT"] = np.ascontiguousarray(
        np.concatenate([audio[samples[j], :, 0:S["V2"][j]] for j in range(NS)], axis=1))
    m["tT"] = np.ascontiguousarray(
        np.asarray(inp["text"])[samples].astype(f16).transpose(2, 0, 1).reshape(300, NS * 30))
    m["vT"] = np.ascontiguousarray(
        np.asarray(inp["video"])[samples].astype(f16).transpose(2, 0, 1).reshape(4096, NS * 16))
    nfr = np.asarray(inp["audio_STFT_nframes"]).astype(np.int64)[samples]
    m["nf"] = np.ascontiguousarray(np.maximum(1, nfr // 16).astype(np.int32)[:, None])

    sl = slice(512 * core, 512 * (core + 1))
    for nm, src in (("gutf", "gut_fw"), ("gutc", "gut_cw"),
                    ("guvf", "guv_fw"), ("guvc", "guv_cw")):
        wT = np.asarray(inp[src]).astype(np.float32).T[:, sl].astype(f16)  # (4096, 512)
        a = wT.reshape(32, 128, 512).transpose(1, 0, 2)
        m[nm + "T"] = np.ascontiguousarray(a.reshape(128, 32 * 512))
        m[nm + "b"] = np.ascontiguousarray(
            np.asarray(inp[src.replace("w", "b")]).astype(f16)[None, sl])
    wT = np.asarray(inp["proj_w"]).astype(np.float32).T[:, sl].astype(f16)  # (1024, 512)
    a = wT.reshape(8, 128, 512).transpose(1, 0, 2)
    m["projT"] = np.ascontiguousarray(a.reshape(128, 8 * 512))
    m["projb"] = np.ascontiguousarray(np.asarray(inp["proj_b"]).astype(f16)[None, sl])
    return m


def assemble_output(results, order):
    """results[c]["out"] is [3, 32, 512] (all samples, this core's 512 cols)."""
    full = np.empty((3, NB, 4096), np.float32)
    inv = np.empty(NB, np.int64)
    for p in range(NB):
        c, j = p // NS, p % NS
        inv[p] = order[NC * j + c]
    for c2 in range(NC):
        full[:, inv, 512 * c2:512 * (c2 + 1)] = results[c2]["out"]
    return full


# ---------------------------------------------------------------------------
# public entry point
# ---------------------------------------------------------------------------
_NC_CACHE = {}


def _get_nc(P=None, debug=False):
    if P is None:
        assert _NC_CACHE, "call kernel() or prepare() first"
        return next(iter(_NC_CACHE.values()))
    key = (P, debug)
    if key not in _NC_CACHE:
        _NC_CACHE[key] = build(P, debug=debug)
    return _NC_CACHE[key]


def prepare(inputs, debug=False):
    order, P = plan_from_inputs(inputs)
    nc = _get_nc(P, debug=debug)
    w = prep_shared(inputs)
    in_maps = [prep_core_inputs(inputs, w, order, P, c) for c in range(NC)]
    return nc, in_maps, order, P


def kernel(**inputs):
    from concourse.bass_utils import run_bass_kernel_spmd

    nc, in_maps, order, P = prepare(inputs)
    res = run_bass_kernel_spmd(nc, in_maps, core_ids=list(range(NC)))
    return assemble_output([res.results[c] for c in range(NC)], order)

